# revision 1
# baseline (speedup 1.0000x reference)
"""Trainium2 Bass kernel for a linear-attention (elu+1 feature map) encoder
layer with SwiGLU projections, distributed over 8 NeuronCores.

Sharding: tokens. B*S = 4*4096 = 16384 tokens flattened; core c owns tokens
[c*2048, (c+1)*2048) == the (c%2)-th half of batch c//2's sequence. All
weights are broadcast. The only cross-core dependency is the linear-attention
state kv = phi_k^T @ [v|1] summed over a full sequence, reduced with a tiny
(128x8x65 fp32) AllReduce over core pairs [[0,1],[2,3],[4,5],[6,7]].

Pass order hides the collective: LN1 -> K/V+kv-accum -> AllReduce issued ->
Q projection (overlaps the collective) -> attention readout + out-proj +
residual -> FFN (with LN2 fused at the head of each token block).

Matmuls run in float32r (fp32 operands rounded to ~13-bit mantissa in the PE
datapath, full rate at even N>=256) except the small attention matmuls
(odd N / nonzero psum base partition are illegal for fp32r) which use fp32,
and the FFN h1 @ w2 matmul which uses bf16 (h1 magnitudes are small vs the
residual stream).

Layout: token-major [tok, feat] for x, k, v, phi_k, attn, x1, out;
feature-major [feat, tok] for x2T, phi_qT, attnT, h1T. Matmul computes
out[M,N] = lhsT.T @ rhs with contraction on the partition dim.
"""

import numpy as np
from contextlib import ExitStack

B, S, D, H, DK, DFF = 4, 4096, 1024, 16, 64, 4096
NCORES = 8
TOK = B * S // NCORES  # 2048 tokens per core
TT = TOK // 128  # 16 token tiles
KT = D // 128  # 8 feature tiles of D
FT = DFF // 128  # 32 feature tiles of DFF
LN_EPS = 1e-5
ATTN_EPS = 1e-6

W_NAMES = [
    "ln1_g", "ln1_b", "ln2_g", "ln2_b",
    "q_w1", "q_b1", "q_w2", "q_b2",
    "k_w1", "k_b1", "k_w2", "k_b2",
    "v_w1", "v_b1", "v_w2", "v_b2",
    "out_w", "out_b",
    "ff_w1", "ff_b1", "ff_w2", "ff_b2", "ff_w3", "ff_b3",
]

_CACHE = {}


def _build():
    import concourse.bass as bass
    import concourse.tile as tile
    from concourse import bacc, mybir
    from concourse.bass import ds, ts
    from concourse.masks import make_identity

    f32 = mybir.dt.float32
    f32r = mybir.dt.float32r
    bf16 = mybir.dt.bfloat16
    Act = mybir.ActivationFunctionType
    Alu = mybir.AluOpType

    nc = bacc.Bacc("TRN2", target_bir_lowering=False, debug=False, num_devices=NCORES)

    # ---- I/O ----
    x_d = nc.dram_tensor("x", [TOK, D], f32, kind="ExternalInput").ap()
    mask_d = nc.dram_tensor("mask", [TOK], f32, kind="ExternalInput").ap()

    wd = {}
    for nm, shape, dt_ in [
        ("ln1_g", [D], f32), ("ln1_b", [D], f32),
        ("ln2_g", [D], f32), ("ln2_b", [D], f32),
        ("q_w1", [D, D], f32r), ("q_b1", [D], f32),
        ("q_w2", [D, D], f32r), ("q_b2", [D], f32),
        ("k_w1", [D, D], f32r), ("k_b1", [D], f32),
        ("k_w2", [D, D], f32r), ("k_b2", [D], f32),
        ("v_w1", [D, D], f32r), ("v_b1", [D], f32),
        ("v_w2", [D, D], f32r), ("v_b2", [D], f32),
        ("out_w", [D, D], f32r), ("out_b", [D], f32r),
        ("ff_w1", [D, DFF], f32r), ("ff_b1", [DFF], f32),
        ("ff_w2", [DFF, D], f32), ("ff_b2", [D], f32),
        ("ff_w3", [D, DFF], f32r), ("ff_b3", [DFF], f32),
    ]:
        wd[nm] = nc.dram_tensor(nm, shape, dt_, kind="ExternalInput").ap()

    out_d = nc.dram_tensor("out", [TOK, D], f32, kind="ExternalOutput").ap()

    # ---- DRAM scratch ----
    phiq_sp = nc.dram_tensor("phiq_sp", [KT, 128, TOK], bf16).ap()
    x1_sp = nc.dram_tensor("x1_sp", [TOK, D], f32).ap()
    rb_d = nc.dram_tensor("rb_d", [4, H, 512], f32).ap()
    kv_in = nc.dram_tensor("kv_in", [128, H // 2, DK + 1], f32).ap()
    kv_out = nc.dram_tensor("kv_out", [128, H // 2, DK + 1], f32).ap()

    def bcast(v, n, offset=0):
        # [n]-vector -> [128, n] partition-broadcast DMA source
        return bass.AP(tensor=v.tensor, offset=v.offset + offset, ap=[[0, 128], [1, n]])

    def wslice(w, col0, ncol):
        # [D_in, N] weight block, cols [col0, col0+ncol) -> [128, D_in//128, ncol]
        return w[:, ds(col0, ncol)].rearrange("(k p) n -> p k n", p=128)

    def ln_tile(pool, xt, eps_t, tag):
        """LayerNorm of token-major [128, D] tile -> new [128, D] f32 tile."""
        stats = pool.tile([128, 2, 6], f32, tag=f"{tag}_st", name=f"{tag}_st")
        nc.vector.bn_stats(out=stats[:, 0, :], in_=xt[:, 0:512])
        nc.vector.bn_stats(out=stats[:, 1, :], in_=xt[:, 512:1024])
        mv = pool.tile([128, 2], f32, tag=f"{tag}_mv", name=f"{tag}_mv")
        nc.vector.bn_aggr(out=mv[:], in_=stats[:])
        sq = pool.tile([128, 1], f32, tag=f"{tag}_sq", name=f"{tag}_sq")
        nc.scalar.activation(sq[:], mv[:, 1:2], Act.Sqrt, bias=eps_t[:], scale=1.0)
        rstd = pool.tile([128, 1], f32, tag=f"{tag}_rs", name=f"{tag}_rs")
        nc.vector.reciprocal(rstd[:], sq[:])
        nmr = pool.tile([128, 1], f32, tag=f"{tag}_nm", name=f"{tag}_nm")
        nc.vector.scalar_tensor_tensor(
            nmr[:], mv[:, 0:1], -1.0, rstd[:], Alu.mult, Alu.mult
        )
        xa = pool.tile([128, D], f32, tag=f"{tag}_xa", name=f"{tag}_xa")
        nc.scalar.activation(xa[:], xt[:], Act.Identity, bias=nmr[:], scale=rstd[:])
        return xa

    with tile.TileContext(nc) as tc, ExitStack() as ctx:
        consts = ctx.enter_context(tc.tile_pool(name="consts", bufs=1))

        ident = consts.tile([128, 128], f32)
        make_identity(nc, ident[:])
        eps_t = consts.tile([128, 1], f32)
        nc.vector.memset(eps_t[:], LN_EPS)
        ones_f = consts.tile([128, 1], f32)
        nc.vector.memset(ones_f[:], 1.0)
        ident_h = consts.tile([128, 128], bf16)
        nc.vector.tensor_copy(ident_h[:], ident[:])
        mask_sb = consts.tile([128, TT], f32)
        nc.sync.dma_start(mask_sb[:], mask_d.rearrange("(t p) -> p t", p=128))
        qb1_sb = consts.tile([128, KT], f32)
        nc.sync.dma_start(qb1_sb[:], wd["q_b1"].rearrange("(k p) -> p k", p=128))
        qb2_sb = consts.tile([128, KT], f32)
        nc.sync.dma_start(qb2_sb[:], wd["q_b2"].rearrange("(k p) -> p k", p=128))
        ffb1_sb = consts.tile([128, FT], f32)
        nc.sync.dma_start(ffb1_sb[:], wd["ff_b1"].rearrange("(k p) -> p k", p=128))
        ffb3_sb = consts.tile([128, FT], f32)
        nc.sync.dma_start(ffb3_sb[:], wd["ff_b3"].rearrange("(k p) -> p k", p=128))
        ln1g_sb = consts.tile([128, KT], f32)
        nc.sync.dma_start(ln1g_sb[:], wd["ln1_g"].rearrange("(k p) -> p k", p=128))
        ln1b_sb = consts.tile([128, KT], f32)
        nc.sync.dma_start(ln1b_sb[:], wd["ln1_b"].rearrange("(k p) -> p k", p=128))
        ln2g_sb = consts.tile([128, KT], f32)
        nc.sync.dma_start(ln2g_sb[:], wd["ln2_g"].rearrange("(k p) -> p k", p=128))
        ln2b_sb = consts.tile([128, KT], f32)
        nc.sync.dma_start(ln2b_sb[:], wd["ln2_b"].rearrange("(k p) -> p k", p=128))
        kv_acc = consts.tile([128, H // 2, DK + 1], f32)
        nc.vector.memset(kv_acc[:], 0.0)

        # out-proj weight pool created early (LIFO: released after pass 4)
        aw_cm = tc.tile_pool(name="aw", bufs=1)
        aw = aw_cm.__enter__()

        # x2T: post-LN1 activations, feature-major, resident through Q pass
        x2t_cm = tc.tile_pool(name="x2tp", bufs=1)
        x2tp = x2t_cm.__enter__()
        x2T = [
            x2tp.tile([128, TOK], f32r, tag=f"x2t{k}", name=f"x2t{k}")
            for k in range(KT)
        ]

        # ================= Pass 1: LN1 + transpose =================
        with (
            tc.tile_pool(name="lnp", bufs=3) as lp,
            tc.tile_pool(name="lnps", bufs=4, space="PSUM") as lps,
        ):
            for t in range(TT):
                xt = lp.tile([128, D], f32, tag="xt")
                nc.sync.dma_start(xt[:], x_d[ts(t, 128), :])
                x2 = ln_tile(lp, xt, eps_t, "l1")
                for k in range(KT):
                    tp = lps.tile([128, 128], f32, tag="tp")
                    nc.tensor.transpose(tp[:], x2[:, ts(k, 128)], ident[:])
                    nc.vector.tensor_scalar(
                        x2T[k][:, ts(t, 128)], tp[:],
                        ln1g_sb[:, k : k + 1], ln1b_sb[:, k : k + 1],
                        Alu.mult, Alu.add,
                    )

        # ========== Pass 2: K/V projections + phi_k + kv accumulation ======
        with (
            tc.tile_pool(name="kvp", bufs=2) as kp,
            tc.tile_pool(name="kvw", bufs=1) as kw,
            tc.tile_pool(name="kvps", bufs=4, space="PSUM") as kps,
            tc.tile_pool(name="kvsps", bufs=4, space="PSUM") as ksp,
        ):
            for blk in range(2):
                wts = {}
                for nm in ("k_w1", "k_w2", "v_w1", "v_w2"):
                    wt = kw.tile([128, KT, 512], f32r, tag=nm, name=f"w_{nm}")
                    nc.sync.dma_start(wt[:], wslice(wd[nm], blk * 512, 512))
                    wts[nm] = wt
                bcs = {}
                for nm in ("k_b1", "k_b2", "v_b1", "v_b2"):
                    bc_ = kw.tile([128, 512], f32, tag=f"bc_{nm}", name=f"bc_{nm}")
                    nc.sync.dma_start(bc_[:], bcast(wd[nm], 512, offset=blk * 512))
                    bcs[nm] = bc_
                for t in range(TT):
                    prj = {}
                    for nm in ("k_w1", "k_w2", "v_w1", "v_w2"):
                        p_ = kps.tile([128, 512], f32, tag="proj", name=f"prj_{nm}")
                        for k in range(KT):
                            nc.tensor.matmul(
                                p_[:],
                                x2T[k][:, ts(t, 128)],
                                wts[nm][:, k, :],
                                start=(k == 0),
                                stop=(k == KT - 1),
                            )
                        prj[nm] = p_
                    k1b = kp.tile([128, 512], f32, tag="pb1", name="k1b")
                    nc.vector.tensor_add(k1b[:], prj["k_w1"][:], bcs["k_b1"][:])
                    s1k = kp.tile([128, 512], f32, tag="psl", name="s1k")
                    nc.scalar.activation(s1k[:], k1b[:], Act.Silu)
                    ksb = kp.tile([128, 512], f32, tag="pb2", name="ksb")
                    nc.vector.tensor_add(ksb[:], prj["k_w2"][:], bcs["k_b2"][:])
                    ksg = kp.tile([128, 512], f32, tag="ksg")
                    nc.vector.tensor_mul(ksg[:], s1k[:], ksb[:])
                    tmpk = kp.tile([128, 512], f32, tag="pb1", name="tmpk")
                    nc.vector.tensor_scalar_min(tmpk[:], ksg[:], 0.0)
                    ek = kp.tile([128, 512], f32, tag="ek")
                    nc.scalar.activation(ek[:], tmpk[:], Act.Exp)
                    phk0 = kp.tile([128, 512], f32, tag="phk0")
                    nc.vector.scalar_tensor_tensor(
                        phk0[:], ksg[:], 0.0, ek[:], Alu.max, Alu.add
                    )
                    phik = kp.tile([128, 512], bf16, tag="phik")
                    nc.vector.tensor_scalar_mul(
                        phik[:], phk0[:], mask_sb[:, t : t + 1]
                    )
                    v1b = kp.tile([128, 512], f32, tag="pb1", name="v1b")
                    nc.vector.tensor_add(v1b[:], prj["v_w1"][:], bcs["v_b1"][:])
                    s1v = kp.tile([128, 512], f32, tag="psl", name="s1v")
                    nc.scalar.activation(s1v[:], v1b[:], Act.Silu)
                    v2b = kp.tile([128, 512], f32, tag="pb2", name="v2b")
                    nc.vector.tensor_add(v2b[:], prj["v_w2"][:], bcs["v_b2"][:])
                    # v_aug = [v | 1] per head: strided write + ones column
                    vr = kp.tile([128, 8, DK + 1], bf16, tag="vr")
                    nc.vector.tensor_mul(vr[:, :, 0:64], s1v[:], v2b[:])
                    nc.vector.memset(vr[:, :, 64:65], 1.0)
                    # kv[h] = phi_k_h^T @ [v_h | 1]  (2 heads packed per psum tile)
                    for hp in range(4):
                        ph = ksp.tile([128, DK + 1], f32, tag="kvps", name="kvps")
                        for sub in range(2):
                            hh = hp * 2 + sub
                            nc.tensor.matmul(
                                ph[ds(sub * 64, 64), :],
                                phik[:, ds(hh * 64, 64)],
                                vr[:, hh, :],
                                start=True,
                                stop=True,
                                tile_position=(0, sub * 64),
                            )
                        gp = blk * 4 + hp
                        nc.vector.tensor_add(
                            kv_acc[:, gp, :], kv_acc[:, gp, :], ph[:]
                        )

        # ======= kv AllReduce over core pairs (overlaps the Q pass) =======
        nc.sync.dma_start(kv_in[:], kv_acc[:])
        nc.gpsimd.collective_compute(
            "AllReduce",
            mybir.AluOpType.add,
            replica_groups=[[0, 1], [2, 3], [4, 5], [6, 7]],
            ins=[kv_in[:]],
            outs=[kv_out[:]],
        )
        kv_f = consts.tile([128, H // 2, DK + 1], f32)
        nc.sync.dma_start(kv_f[:], kv_out[:])
        kv_h = consts.tile([128, H // 2, DK + 1], bf16)
        nc.vector.tensor_copy(kv_h[:], kv_f[:])

        # out-proj weights prefetch (overlaps pass 3)
        outw_sb = aw.tile([128, KT, D], f32r)
        nc.sync.dma_start(outw_sb[:], wslice(wd["out_w"], 0, D))
        outb_row = aw.tile([1, D], f32r)
        nc.sync.dma_start(outb_row[:], wd["out_b"][None, :])
        onesrow_r = aw.tile([1, 128], f32r)
        onesrow_f = aw.tile([1, 128], f32)
        nc.vector.memset(onesrow_f[:], 1.0)
        nc.vector.tensor_copy(onesrow_r[:], onesrow_f[:])

        # ============ Pass 3: Q projection + phi_q -> DRAM spill =========
        with (
            tc.tile_pool(name="qp", bufs=2) as qp,
            tc.tile_pool(name="qw", bufs=1) as qw,
            tc.tile_pool(name="qps", bufs=3, space="PSUM") as qps,
        ):
            for blk in range(2):
                qw1b = qw.tile([128, KT, 512], f32r, tag="qw1")
                nc.sync.dma_start(qw1b[:], wslice(wd["q_w1"], blk * 512, 512))
                qw2b = qw.tile([128, KT, 512], f32r, tag="qw2")
                nc.sync.dma_start(qw2b[:], wslice(wd["q_w2"], blk * 512, 512))
                for dk in range(4):
                    dout_k = blk * 4 + dk
                    for tb in range(4):
                        ps1 = qps.tile([128, 512], f32, tag="ps1")
                        ps2 = qps.tile([128, 512], f32, tag="ps2")
                        for k in range(KT):
                            nc.tensor.matmul(
                                ps1[:],
                                qw1b[:, k, ds(dk * 128, 128)],
                                x2T[k][:, ds(tb * 512, 512)],
                                start=(k == 0),
                                stop=(k == KT - 1),
                            )
                        for k in range(KT):
                            nc.tensor.matmul(
                                ps2[:],
                                qw2b[:, k, ds(dk * 128, 128)],
                                x2T[k][:, ds(tb * 512, 512)],
                                start=(k == 0),
                                stop=(k == KT - 1),
                            )
                        s1 = qp.tile([128, 512], f32, tag="s1")
                        nc.scalar.activation(
                            s1[:], ps1[:], Act.Silu,
                            bias=qb1_sb[:, dout_k : dout_k + 1], scale=1.0,
                        )
                        qt = qp.tile([128, 512], f32, tag="qt")
                        nc.vector.scalar_tensor_tensor(
                            qt[:], ps2[:], qb2_sb[:, dout_k : dout_k + 1], s1[:],
                            Alu.add, Alu.mult,
                        )
                        tmp = qp.tile([128, 512], f32, tag="tmp")
                        nc.vector.tensor_scalar_min(tmp[:], qt[:], 0.0)
                        e = qp.tile([128, 512], f32, tag="e")
                        nc.scalar.activation(e[:], tmp[:], Act.Exp)
                        phq = qp.tile([128, 512], bf16, tag="phq")
                        nc.vector.scalar_tensor_tensor(
                            phq[:], qt[:], 0.0, e[:], Alu.max, Alu.add
                        )
                        nc.sync.dma_start(
                            phiq_sp[dout_k, :, ds(tb * 512, 512)], phq[:]
                        )
        x2t_cm.__exit__(None, None, None)

        # ===== Pass 4: attention readout + out-proj + residual =====
        with (
            tc.tile_pool(name="ap", bufs=2) as ap,
            tc.tile_pool(name="anum", bufs=4, space="PSUM") as anum,
            tc.tile_pool(name="adnm", bufs=2, space="PSUM") as adnm,
            tc.tile_pool(name="aops", bufs=2, space="PSUM") as aops,
            tc.tile_pool(name="aps2", bufs=2, space="PSUM") as aps2,
        ):
            for c in range(4):  # 512-token chunks
                col = ds(c * 512, 512)
                pq = []
                for k in range(KT):
                    pqk = ap.tile([128, 512], bf16, tag=f"pq{k}", name=f"pq{k}")
                    nc.sync.dma_start(pqk[:], phiq_sp[k, :, col])
                    pq.append(pqk)
                # Stage 1: all num/denom matmuls; denom rows stream to DRAM.
                # Stage 2: broadcast-reload + divide. Two loops so the DRAM
                # round-trip latency overlaps the remaining matmul work.
                nsbs = []
                for hp in range(KT):
                    nps = anum.tile([128, 512], f32, tag="num")
                    for sub in range(2):
                        nc.tensor.matmul(
                            nps[ds(sub * 64, 64), :],
                            kv_h[ds(sub * 64, 64), hp, 0:64].opt(),
                            pq[hp][ds(sub * 64, 64), :],
                            start=True,
                            stop=True,
                            tile_position=(sub * 64, sub * 64),
                        )
                        dn = adnm.tile([1, 512], f32, tag="dnum")
                        nc.tensor.matmul(
                            dn[:],
                            kv_h[ds(sub * 64, 64), hp, 64:65].opt(),
                            pq[hp][ds(sub * 64, 64), :],
                            start=True,
                            stop=True,
                            tile_position=(sub * 64, 0),
                        )
                        dsb = ap.tile([1, 512], f32, tag=f"dsb{sub}", name=f"dsb{sub}")
                        nc.scalar.copy(dsb[:], dn[:])
                        nc.sync.dma_start(rb_d[c, 2 * hp + sub, :], dsb[:])
                    nsb = ap.tile([128, 512], f32, tag=f"nsb{hp}", name=f"nsb{hp}")
                    nc.scalar.copy(nsb[:], nps[:])
                    nsbs.append(nsb)
                aT = []
                for hp in range(KT):
                    rbc = ap.tile([128, 512], f32, tag="rbc")
                    nc.sync.dma_start(
                        rbc[:],
                        bass.AP(
                            tensor=rb_d.tensor,
                            offset=rb_d.offset + (c * H + 2 * hp) * 512,
                            ap=[[512, 2], [0, 64], [1, 512]],
                        ),
                    )
                    rbe = ap.tile([128, 512], f32, tag="rbe")
                    nc.vector.tensor_scalar_add(rbe[:], rbc[:], ATTN_EPS)
                    rbr = ap.tile([128, 512], f32, tag="rbr")
                    nc.vector.reciprocal(rbr[:], rbe[:])
                    aTk = ap.tile([128, 512], f32r, tag=f"aT{hp}", name=f"aT{hp}")
                    nc.vector.scalar_tensor_tensor(
                        aTk[:], nsbs[hp][:], 0.0, rbr[:], Alu.add, Alu.mult
                    )
                    aT.append(aTk)
                # out-proj + residual per 128-token subtile
                for tsub in range(4):
                    t = c * 4 + tsub
                    xt = ap.tile([128, D], f32, tag="xres")
                    nc.sync.dma_start(xt[:], x_d[ts(t, 128), :])
                    x1 = ap.tile([128, D], f32, tag="x1")
                    for dh in range(2):
                        op_ = aops.tile([128, 512], f32, tag="ops")
                        for k in range(KT):
                            nc.tensor.matmul(
                                op_[:],
                                aT[k][:, ts(tsub, 128)],
                                outw_sb[:, k, ds(dh * 512, 512)],
                                start=(k == 0),
                                stop=False,
                            )
                        nc.tensor.matmul(
                            op_[:],
                            onesrow_r[:],
                            outb_row[:, ds(dh * 512, 512)],
                            start=False,
                            stop=True,
                        )
                        nc.vector.tensor_add(
                            x1[:, ds(dh * 512, 512)], op_[:], xt[:, ds(dh * 512, 512)]
                        )
                    nc.sync.dma_start(x1_sp[ts(t, 128), :], x1[:])
        aw_cm.__exit__(None, None, None)

        # ========= Pass 5: LN2 + SwiGLU FFN + residual =========
        with (
            tc.tile_pool(name="fp", bufs=2) as fp,
            tc.tile_pool(name="fl", bufs=2) as fl,
            tc.tile_pool(name="fw", bufs=3) as fw,
            tc.tile_pool(name="fw2", bufs=1) as fw2,
            tc.tile_pool(name="fh", bufs=1) as fh,
            tc.tile_pool(name="fc", bufs=1) as fc,
            tc.tile_pool(name="fps", bufs=2, space="PSUM") as fps,
            tc.tile_pool(name="fps2", bufs=2, space="PSUM") as fps2,
            tc.tile_pool(name="fps3", bufs=2, space="PSUM") as fps3,
        ):
            ffb2_bc = fc.tile([128, D], f32)
            nc.sync.dma_start(ffb2_bc[:], bcast(wd["ff_b2"], D))
            for tb in range(2):  # 1024-token blocks
                # LN2 + transpose for this block's 8 token tiles
                x2Tb = [
                    fh.tile([128, 1024], f32r, tag=f"x2b{k}", name=f"x2b{k}")
                    for k in range(KT)
                ]
                for tt_ in range(8):
                    t = tb * 8 + tt_
                    x1t = fl.tile([128, D], f32, tag="x1t")
                    nc.sync.dma_start(x1t[:], x1_sp[ts(t, 128), :])
                    x2f = ln_tile(fl, x1t, eps_t, "l2")
                    for k in range(KT):
                        tp = fps3.tile([128, 128], f32, tag="tp3")
                        nc.tensor.transpose(tp[:], x2f[:, ts(k, 128)], ident[:])
                        nc.vector.tensor_scalar(
                            x2Tb[k][:, ts(tt_, 128)], tp[:],
                            ln2g_sb[:, k : k + 1], ln2b_sb[:, k : k + 1],
                            Alu.mult, Alu.add,
                        )
                h1 = [
                    fh.tile([128, 1024], bf16, tag=f"h1_{j}", name=f"h1_{j}")
                    for j in range(FT)
                ]
                for j in range(FT):
                    w1b = fw.tile([128, KT, 128], f32r, tag="w1b")
                    nc.sync.dma_start(w1b[:], wslice(wd["ff_w1"], j * 128, 128))
                    w3b = fw.tile([128, KT, 128], f32r, tag="w3b")
                    nc.sync.dma_start(w3b[:], wslice(wd["ff_w3"], j * 128, 128))
                    for ch in range(2):
                        p1 = fps.tile([128, 512], f32, tag="p1")
                        p3 = fps.tile([128, 512], f32, tag="p3")
                        for k in range(KT):
                            nc.tensor.matmul(
                                p1[:],
                                w1b[:, k, :],
                                x2Tb[k][:, ds(ch * 512, 512)],
                                start=(k == 0),
                                stop=(k == KT - 1),
                            )
                        for k in range(KT):
                            nc.tensor.matmul(
                                p3[:],
                                w3b[:, k, :],
                                x2Tb[k][:, ds(ch * 512, 512)],
                                start=(k == 0),
                                stop=(k == KT - 1),
                            )
                        s1 = fp.tile([128, 512], f32, tag="fs1")
                        nc.scalar.activation(
                            s1[:], p1[:], Act.Silu, bias=ffb1_sb[:, j : j + 1],
                            scale=1.0,
                        )
                        nc.vector.scalar_tensor_tensor(
                            h1[j][:, ds(ch * 512, 512)],
                            p3[:],
                            ffb3_sb[:, j : j + 1],
                            s1[:],
                            Alu.add,
                            Alu.mult,
                        )
                for dh in range(2):
                    w2s = []
                    for j in range(FT):
                        stg = fw.tile([128, 512], f32, tag="w2stg")
                        nc.sync.dma_start(
                            stg[:], wd["ff_w2"][ts(j, 128), ds(dh * 512, 512)]
                        )
                        w2j = fw2.tile([128, 512], bf16, tag=f"w2_{j}", name=f"w2_{j}")
                        nc.vector.tensor_copy(w2j[:], stg[:])
                        w2s.append(w2j)
                    for tsub in range(8):
                        op_ = fps2.tile([128, 512], f32, tag="op")
                        for j in range(FT):
                            nc.tensor.matmul(
                                op_[:],
                                h1[j][:, ts(tsub, 128)],
                                w2s[j][:],
                                start=(j == 0),
                                stop=(j == FT - 1),
                            )
                        row0 = tb * 1024 + tsub * 128
                        of = fp.tile([128, 512], f32, tag="of")
                        nc.vector.tensor_add(
                            of[:], op_[:], ffb2_bc[:, ds(dh * 512, 512)]
                        )
                        x1s = fp.tile([128, 512], f32, tag="x1s")
                        nc.sync.dma_start(
                            x1s[:], x1_sp[ds(row0, 128), ds(dh * 512, 512)]
                        )
                        ot = fp.tile([128, 512], f32, tag="ot")
                        nc.vector.tensor_add(ot[:], of[:], x1s[:])
                        nc.sync.dma_start(
                            out_d[ds(row0, 128), ds(dh * 512, 512)], ot[:]
                        )

    nc.compile()
    return nc


def _get_nc():
    if "nc" not in _CACHE:
        _CACHE["nc"] = _build()
    return _CACHE["nc"]


def kernel(**inputs) -> np.ndarray:
    from concourse.bass_utils import run_bass_kernel_spmd

    nc = _get_nc()
    x = np.ascontiguousarray(np.asarray(inputs["x"], dtype=np.float32))
    mask = np.ascontiguousarray(np.asarray(inputs["mask"], dtype=np.float32))
    x_flat = x.reshape(B * S, D)
    m_flat = mask.reshape(B * S)
    weights = {
        nm: np.ascontiguousarray(np.asarray(inputs[nm], dtype=np.float32))
        for nm in W_NAMES
    }
    in_maps = []
    for c in range(NCORES):
        m = {"x": x_flat[c * TOK : (c + 1) * TOK], "mask": m_flat[c * TOK : (c + 1) * TOK]}
        m.update(weights)
        in_maps.append(m)
    res = run_bass_kernel_spmd(nc, in_maps, list(range(NCORES)))
    out = np.concatenate([res.results[c]["out"] for c in range(NCORES)], axis=0)
    return out.reshape(B, S, D)

